# revision 2
# baseline (speedup 1.0000x reference)
"""Trainium2 Bass kernel for nn_ContextAwareModel (2-layer 'bidirectional' LSTM
over gathered sentence embeddings + dense head).

Strategy (hardcoded, self-contained):
  - Host: gather embedding rows (sparse lookup of 4096+32 rows from the 307MB
    table stays on host - only 12.6MB of gathered data ships to devices),
    pre-transpose/pack all tensors into PE-friendly tiles.
  - Device (SPMD over 8 cores, data-parallel over batch, b=4 per core):
    L0 input projections (big matmuls) -> L0 recurrence (128 steps, both
    directions) -> L1 projections -> L1 recurrence -> dense+classifier.
    All matmul inputs bf16 (FWL weight loads), fp32 PSUM/state.
  - Layout: gates on partitions [128 = hidden-in-chunk], batch on free dim.
    Gate-type order (i, f, o, g) so one sigmoid covers a contiguous slice.
"""

import numpy as np
import ml_dtypes

E = 768
H = 256
B = 32
T = 128
D = E + 2 * H
HALF = D // 2
NCORES = 8
BSH = B // NCORES          # batch per core
U = 2                      # directions per layer
KT0 = E // 128             # 6  K-tiles for L0 input proj
KT1 = (2 * H) // 128       # 4  K-tiles for L1 input proj
KH = H // 128              # 2  K-tiles for recurrent matmul
MT = (4 * H) // 128        # 8  M-tiles of gates per direction
KD = D // 128              # 10 K-tiles for dense
MD = HALF // 128           # 5  M-tiles for dense
# our gate-type order (i,f,o,g) -> pytorch row-block (i,f,g,o)
TP_MAP = [0, 1, 3, 2]

BF16 = ml_dtypes.bfloat16


def _perm_rows():
    """Row permutation: our M-tile m covers pytorch rows TP_MAP[m//2]*256+(m%2)*128."""
    idx = []
    for m in range(MT):
        base = TP_MAP[m // 2] * 256 + (m % 2) * 128
        idx.append(np.arange(base, base + 128))
    return np.concatenate(idx)          # [1024]


_PERM = _perm_rows()


def pack_lstm(w_ih, w_hh, b_ih, b_hh, l1: bool):
    """Pack one layer's weights.
    Returns wi [128,U,KT,MT,128] bf16, wh [128,U,KH,MT,128] bf16, bias [128,U*MT] f32.
    lhsT tile convention: tile[p, q] = W[row(m,q), col(k,p)]  (pre-transposed).
    For L1 the input features are ordered (k_src, u_src, p) to match the
    on-chip h-history layout, i.e. col(j,p) = u_src*256 + k_src*128 + p with
    j = k_src*2 + u_src.
    """
    wp = w_ih[:, _PERM, :]              # [U,1024,IN]
    KT = wp.shape[2] // 128
    if not l1:
        wi = wp.reshape(U, MT, 128, KT, 128).transpose(4, 0, 3, 1, 2)
    else:
        # IN=512 -> [u_src,k_src,p]; want K-tile j=(k_src,u_src)
        t = wp.reshape(U, MT, 128, 2, 2, 128)      # [u,m,q,u_src,k_src,p]
        t = t.transpose(5, 0, 4, 3, 1, 2)          # [p,u,k_src,u_src,m,q]
        wi = t.reshape(128, U, KT, MT, 128)
    whp = w_hh[:, _PERM, :]             # [U,1024,256]
    wh = whp.reshape(U, MT, 128, KH, 128).transpose(4, 0, 3, 1, 2)
    bias = (b_ih + b_hh)[:, _PERM]      # [U,1024]
    bias = bias.reshape(U, MT, 128).transpose(2, 0, 1).reshape(128, U * MT)
    return (np.ascontiguousarray(wi, dtype=BF16),
            np.ascontiguousarray(wh, dtype=BF16),
            np.ascontiguousarray(bias, dtype=np.float32))


def pack_dense(dense_w, dense_b, clf_w, clf_b):
    """dense rep feature order: [target 0:768 natural] + h1 tiles j=(k_src,u_src):
    col = 768 + u_src*256 + k_src*128 + p."""
    cols = []
    for kt in range(KD):
        p = np.arange(128)
        if kt < KT0:
            cols.append(kt * 128 + p)
        else:
            j = kt - KT0
            k_src, u_src = j // 2, j % 2
            cols.append(E + u_src * 256 + k_src * 128 + p)
    COL = np.stack(cols)                               # [10,128]
    dwT = dense_w.T[COL]                               # [10,128,640]
    dw = dwT.reshape(KD, 128, MD, 128).transpose(1, 0, 2, 3)   # [128,10,5,128]
    db = dense_b.reshape(MD, 128).T                    # [128,5]
    cw = clf_w.T.reshape(MD, 128, 2).transpose(1, 0, 2)        # [128,5,2]
    cb = clf_b.reshape(2, 1)
    return (np.ascontiguousarray(dw, dtype=BF16),
            np.ascontiguousarray(db, dtype=np.float32),
            np.ascontiguousarray(cw, dtype=BF16),
            np.ascontiguousarray(cb, dtype=np.float32))


def host_pack(inputs):
    """All host-side preprocessing. Returns (per_core_inmaps_data, target_reps)."""
    emb = np.asarray(inputs["emb_table"], dtype=np.float32)
    article = np.asarray(inputs["article"])
    positions = np.asarray(inputs["positions"])

    looked = emb[article]                              # [B,T,E] fp32
    tgt = emb[article[np.arange(B), positions]]        # [B,E] fp32 (exact output)

    w0i, w0h, b0 = pack_lstm(np.asarray(inputs["w_ih0"], np.float32),
                             np.asarray(inputs["w_hh0"], np.float32),
                             np.asarray(inputs["b_ih0"], np.float32),
                             np.asarray(inputs["b_hh0"], np.float32), l1=False)
    w1i, w1h, b1 = pack_lstm(np.asarray(inputs["w_ih1"], np.float32),
                             np.asarray(inputs["w_hh1"], np.float32),
                             np.asarray(inputs["b_ih1"], np.float32),
                             np.asarray(inputs["b_hh1"], np.float32), l1=True)
    dw, db, cw, cb = pack_dense(np.asarray(inputs["dense_w"], np.float32),
                                np.asarray(inputs["dense_b"], np.float32),
                                np.asarray(inputs["clf_w"], np.float32),
                                np.asarray(inputs["clf_b"], np.float32))

    per_core = []
    for c in range(NCORES):
        bs = slice(c * BSH, (c + 1) * BSH)
        arr = looked[bs]                               # [b,T,768]
        xsT = arr.reshape(BSH, T, KT0, 128).transpose(3, 2, 1, 0)  # [128,6,T,b]
        tgtT = tgt[bs].reshape(BSH, KT0, 128).transpose(2, 1, 0)   # [128,6,b]
        per_core.append({
            "xsT": np.ascontiguousarray(xsT, dtype=BF16),
            "tgtT": np.ascontiguousarray(tgtT, dtype=BF16),
            "w0i": w0i, "w0h": w0h, "b0": b0,
            "w1i": w1i, "w1h": w1h, "b1": b1,
            "dw": dw, "db": db, "cw": cw, "cb": cb,
        })
    return per_core, tgt


# ---------------------------------------------------------------------------
# numpy simulation of the exact device dataflow (for fast host-side checking)
# ---------------------------------------------------------------------------
def simulate(inputs):
    per_core, tgt = host_pack(inputs)
    logits = np.zeros((B, 2), np.float32)
    probs = np.zeros((B, 2), np.float32)
    for c in range(NCORES):
        d = {k: np.asarray(v, np.float32) for k, v in per_core[c].items()}
        xsT = d["xsT"]                                 # [128,6,T,b]

        def proj(wi, KT, rhs_fn, bias):
            # returns projbig [128, U*MT(tcu), T, b] fp32
            out = np.zeros((128, U * MT, T, BSH), np.float32)
            for u in range(U):
                for m in range(MT):
                    acc = np.zeros((128, T * BSH), np.float32)
                    for k in range(KT):
                        lhsT = d[wi][:, u, k, m, :]    # [128,128] = [K,M]
                        rhs = rhs_fn(k)                # [128, T*b]
                        acc += lhsT.T @ rhs
                    out[:, m * U + u] = (acc + bias[:, u * MT + m][:, None]).reshape(128, T, BSH)
            return out

        def rec(projbig, wh):
            hist = np.zeros((128, T, KH, U, BSH), np.float32)
            ct = np.zeros((128, KH, U, BSH), np.float32)
            for t in range(T):
                psG = np.zeros((128, U * MT, BSH), np.float32)
                for u in range(U):
                    for m in range(MT):
                        for k in range(KH):
                            lhsT = d[wh][:, u, k, m, :]
                            rhs = (np.zeros((128, BSH), np.float32) if t == 0
                                   else hist[:, t - 1, k, u, :])
                            psG[:, m * U + u] += lhsT.T @ rhs
                gs = psG + projbig[:, :, t, :]
                gs4 = gs.reshape(128, 4, KH * U, BSH)      # [p, type, (chunk,unit), b]
                i_s = 1 / (1 + np.exp(-gs4[:, 0]))
                f_s = 1 / (1 + np.exp(-gs4[:, 1]))
                o_s = 1 / (1 + np.exp(-gs4[:, 2]))
                g_t = np.tanh(gs4[:, 3])
                ctf = ct.reshape(128, KH * U, BSH)
                ctf[:] = f_s * ctf + i_s * g_t
                hist[:, t] = (o_s * np.tanh(ctf)).reshape(128, KH, U, BSH)
            return hist

        proj0 = proj("w0i", KT0, lambda k: xsT[:, k].reshape(128, T * BSH), d["b0"])
        hist0 = rec(proj0, "w0h")
        proj1 = proj("w1i", KT1,
                     lambda j: hist0[:, :, j // 2, j % 2, :].reshape(128, T * BSH),
                     d["b1"])
        hist1 = rec(proj1, "w1h")

        # dense
        psD = np.zeros((128, MD, BSH), np.float32)
        for mt in range(MD):
            for kt in range(KD):
                lhsT = d["dw"][:, kt, mt, :]
                rhs = (d["tgtT"][:, kt, :] if kt < KT0
                       else hist1[:, T - 1, (kt - KT0) // 2, (kt - KT0) % 2, :])
                psD[:, mt] += lhsT.T @ rhs
        feats = np.tanh(psD + d["db"].reshape(128, MD, 1))
        psL = np.zeros((2, BSH), np.float32)
        for kt in range(MD):
            psL += d["cw"][:, kt, :].T @ feats[:, kt, :]
        lg = psL + d["cb"]
        logits[c * BSH:(c + 1) * BSH] = lg.T
        probs[c * BSH:(c + 1) * BSH] = (1 / (1 + np.exp(-lg))).T
    return logits, probs, tgt


# ---------------------------------------------------------------------------
# Bass program
# ---------------------------------------------------------------------------
_PROG_CACHE = {}


def _build_program():
    if "nc" in _PROG_CACHE:
        return _PROG_CACHE["nc"]
    import concourse.bass as bass
    import concourse.tile as tile
    from concourse import bacc, mybir

    f32 = mybir.dt.float32
    bf16 = mybir.dt.bfloat16
    AF = mybir.ActivationFunctionType

    nc = bacc.Bacc("TRN2", target_bir_lowering=False, debug=False,
                   num_devices=NCORES)

    dr = {}
    def din(name, shape, dt):
        dr[name] = nc.dram_tensor(name, list(shape), dt, kind="ExternalInput").ap()

    din("xsT", (128, KT0, T, BSH), bf16)
    din("tgtT", (128, KT0, BSH), bf16)
    din("w0i", (128, U, KT0, MT, 128), bf16)
    din("w0h", (128, U, KH, MT, 128), bf16)
    din("b0", (128, U * MT), f32)
    din("w1i", (128, U, KT1, MT, 128), bf16)
    din("w1h", (128, U, KH, MT, 128), bf16)
    din("b1", (128, U * MT), f32)
    din("dw", (128, KD, MD, 128), bf16)
    din("db", (128, MD), f32)
    din("cw", (128, MD, 2), bf16)
    din("cb", (2, 1), f32)
    out_lg = nc.dram_tensor("out_logitsT", [2, BSH], f32, kind="ExternalOutput").ap()
    out_pr = nc.dram_tensor("out_probsT", [2, BSH], f32, kind="ExternalOutput").ap()

    with tile.TileContext(nc) as tc:
        with tc.tile_pool(name="const", bufs=1) as cp, \
             tc.tile_pool(name="projp", bufs=1) as projp, \
             tc.tile_pool(name="ew", bufs=3) as ew, \
             tc.tile_pool(name="pp", bufs=2, space=bass.MemorySpace.PSUM) as pp, \
             tc.tile_pool(name="gp", bufs=2, space=bass.MemorySpace.PSUM) as gp:

            sb = {}
            for name, ap in dr.items():
                t_ = cp.tile(list(ap.shape), ap.dtype, tag=name)
                nc.sync.dma_start(t_[:], ap[:])
                sb[name] = t_

            z = cp.tile([128, KH, U, BSH], bf16, tag="zeros")
            nc.gpsimd.memset(z[:], 0.0)

            proj0 = projp.tile([128, U * MT, T, BSH], f32, tag="proj0")
            proj1 = projp.tile([128, U * MT, T, BSH], f32, tag="proj1")
            hist0 = projp.tile([128, T, KH, U, BSH], bf16, tag="hist0")
            hist1 = projp.tile([128, T, KH, U, BSH], bf16, tag="hist1")
            c0 = projp.tile([128, KH * U, BSH], f32, tag="c0")
            c1 = projp.tile([128, KH * U, BSH], f32, tag="c1")
            nc.gpsimd.memset(c0[:], 0.0)
            nc.gpsimd.memset(c1[:], 0.0)

        # ---- input projections ----
            def do_proj(wi_name, KT, rhs_fn, bias_name, projbig):
                wi = sb[wi_name]
                for u in range(U):
                    for m in range(MT):
                        ps = pp.tile([128, T, BSH], f32, tag="pproj")
                        for k in range(KT):
                            nc.tensor.matmul(ps[:], wi[:, u, k, m, :], rhs_fn(k),
                                             start=(k == 0), stop=(k == KT - 1))
                        nc.scalar.activation(
                            projbig[:, m * U + u, :, :], ps[:], AF.Identity,
                            bias=sb[bias_name][:, u * MT + m: u * MT + m + 1])

            do_proj("w0i", KT0, lambda k: sb["xsT"][:, k, :, :], "b0", proj0)

            # ---- recurrence ----
            def do_rec(projbig, wh_name, hist, ct):
                wh = sb[wh_name]
                for t in range(T):
                    psG = gp.tile([128, U * MT, BSH], f32, tag="psG")
                    for u in range(U):
                        for m in range(MT):
                            for k in range(KH):
                                rhs = (z[:, k, u, :] if t == 0
                                       else hist[:, t - 1, k, u, :])
                                nc.tensor.matmul(psG[:, m * U + u, :],
                                                 wh[:, u, k, m, :], rhs,
                                                 start=(k == 0), stop=(k == KH - 1))
                    gs = ew.tile([128, U * MT, BSH], f32, tag="gs")
                    nc.vector.tensor_add(gs[:], psG[:], projbig[:, :, t, :])
                    # type-major slices: i=0:4, f=4:8, o=8:12, g=12:16 (tcu blocks)
                    sig = ew.tile([128, 3 * KH * U, BSH], f32, tag="sig")
                    nc.scalar.activation(sig[:], gs[:, 0:3 * KH * U, :], AF.Sigmoid)
                    gt = ew.tile([128, KH * U, BSH], f32, tag="gt")
                    nc.scalar.activation(gt[:], gs[:, 3 * KH * U:, :], AF.Tanh)
                    t1 = ew.tile([128, KH * U, BSH], f32, tag="t1")
                    nc.vector.tensor_mul(t1[:], sig[:, 0:KH * U, :], gt[:])
                    nc.vector.tensor_mul(ct[:], ct[:], sig[:, KH * U:2 * KH * U, :])
                    nc.vector.tensor_add(ct[:], ct[:], t1[:])
                    tc2 = ew.tile([128, KH * U, BSH], f32, tag="tc2")
                    nc.scalar.activation(tc2[:], ct[:], AF.Tanh)
                    nc.vector.tensor_mul(hist[:, t, :, :, :],
                                         sig[:, 2 * KH * U:3 * KH * U, :], tc2[:])

            do_rec(proj0, "w0h", hist0, c0)
            do_proj("w1i", KT1,
                    lambda j: hist0[:, :, j // 2, j % 2, :], "b1", proj1)
            do_rec(proj1, "w1h", hist1, c1)

            # ---- dense + classifier ----
            psD = gp.tile([128, MD, BSH], f32, tag="psD")
            for mt in range(MD):
                for kt in range(KD):
                    rhs = (sb["tgtT"][:, kt, :] if kt < KT0
                           else hist1[:, T - 1, (kt - KT0) // 2, (kt - KT0) % 2, :])
                    nc.tensor.matmul(psD[:, mt, :], sb["dw"][:, kt, mt, :], rhs,
                                     start=(kt == 0), stop=(kt == KD - 1))
            feats = ew.tile([128, MD, BSH], bf16, tag="feats")
            for mt in range(MD):
                nc.scalar.activation(feats[:, mt, :], psD[:, mt, :], AF.Tanh,
                                     bias=sb["db"][:, mt:mt + 1])
            psL = gp.tile([2, BSH], f32, tag="psL")
            for kt in range(MD):
                nc.tensor.matmul(psL[:], sb["cw"][:, kt, :], feats[:, kt, :],
                                 start=(kt == 0), stop=(kt == MD - 1))
            lgt = ew.tile([2, BSH], f32, tag="lgt")
            nc.scalar.activation(lgt[:], psL[:], AF.Identity, bias=sb["cb"][:, 0:1])
            prt = ew.tile([2, BSH], f32, tag="prt")
            nc.scalar.activation(prt[:], psL[:], AF.Sigmoid, bias=sb["cb"][:, 0:1])
            nc.sync.dma_start(out_lg[:], lgt[:])
            nc.sync.dma_start(out_pr[:], prt[:])

    nc.compile()
    _PROG_CACHE["nc"] = nc
    return nc


def kernel(**inputs):
    per_core, tgt = host_pack(inputs)
    nc = _build_program()
    from concourse.bass_utils import run_bass_kernel_spmd
    res = run_bass_kernel_spmd(nc, per_core, core_ids=list(range(NCORES)),
                               trace=False)
    logits = np.zeros((B, 2), np.float32)
    probs = np.zeros((B, 2), np.float32)
    for c in range(NCORES):
        logits[c * BSH:(c + 1) * BSH] = res.results[c]["out_logitsT"].T
        probs[c * BSH:(c + 1) * BSH] = res.results[c]["out_probsT"].T
    return logits, probs, tgt.astype(np.float32)


# revision 8
# speedup vs baseline: 1.0279x; 1.0279x over previous
"""Trainium2 Bass kernel for nn_ContextAwareModel (2-layer 'bidirectional' LSTM
over gathered sentence embeddings + dense head).

Strategy (hardcoded, self-contained):
  - Host: gather embedding rows (sparse lookup of 4096+32 rows from the 307MB
    table stays on host - only 12.6MB of gathered data ships to devices),
    pre-transpose/pack all tensors into PE-friendly tiles.
  - Device (SPMD over 8 cores, data-parallel over batch, b=4 per core):
    L0 input projections (big matmuls) -> L0 recurrence (128 steps, both
    directions) -> L1 projections -> L1 recurrence -> dense+classifier.
    All matmul inputs bf16 (FWL weight loads), fp32 PSUM/state.
  - Layout: gates on partitions [128 = hidden-in-chunk], batch on free dim.
    Gate-type order (i, f, o, g) so one sigmoid covers a contiguous slice.
"""

import numpy as np
import ml_dtypes

E = 768
H = 256
B = 32
T = 128
D = E + 2 * H
HALF = D // 2
NCORES = 8
BSH = B // NCORES          # batch per core
U = 2                      # directions per layer
KT0 = E // 128             # 6  K-tiles for L0 input proj
KT1 = (2 * H) // 128       # 4  K-tiles for L1 input proj
KH = H // 128              # 2  K-tiles for recurrent matmul
MT = (4 * H) // 128        # 8  M-tiles of gates per direction
KD = D // 128              # 10 K-tiles for dense
MD = HALF // 128           # 5  M-tiles for dense
# our gate-type order (i,f,o,g) -> pytorch row-block (i,f,g,o)
TP_MAP = [0, 1, 3, 2]

BF16 = ml_dtypes.bfloat16


def _perm_rows():
    """Row permutation: our M-tile m covers pytorch rows TP_MAP[m//2]*256+(m%2)*128."""
    idx = []
    for m in range(MT):
        base = TP_MAP[m // 2] * 256 + (m % 2) * 128
        idx.append(np.arange(base, base + 128))
    return np.concatenate(idx)          # [1024]


_PERM = _perm_rows()


def pack_lstm(w_ih, w_hh, b_ih, b_hh, l1: bool):
    """Pack one layer's weights.
    Returns wi [128,U,KT,MT,128] bf16, wh [128,U,KH,MT,128] bf16, bias [128,U*MT] f32.
    lhsT tile convention: tile[p, q] = W[row(m,q), col(k,p)]  (pre-transposed).
    For L1 the input features are ordered (k_src, u_src, p) to match the
    on-chip h-history layout, i.e. col(j,p) = u_src*256 + k_src*128 + p with
    j = k_src*2 + u_src.
    """
    # Gate-row scaling for the fused one-tanh cell update:
    #   sigmoid(x) = (tanh(x/2)+1)/2  -> scale i,f,o gate rows by 0.5 so one
    #   Tanh activation serves all four gates.
    #   h is stored as h2 = 2h -> scale all h-consuming weight columns by 0.5
    #   (w_hh here; w_ih1 and dense h-columns elsewhere).
    gs_row = np.ones((4 * H, 1), np.float32)
    for m in range(MT):
        if m // 2 < 3:                  # our types i,f,o
            gs_row[m * 128:(m + 1) * 128] = 0.5
    wp = w_ih[:, _PERM, :] * gs_row     # [U,1024,IN]
    if l1:
        wp = wp * 0.5                   # consumes h2 from layer 0
    KT = wp.shape[2] // 128
    if not l1:
        wi = wp.reshape(U, MT, 128, KT, 128).transpose(4, 0, 3, 1, 2)
    else:
        # IN=512 -> [u_src,k_src,p]; want K-tile j=(k_src,u_src)
        t = wp.reshape(U, MT, 128, 2, 2, 128)      # [u,m,q,u_src,k_src,p]
        t = t.transpose(5, 0, 4, 3, 1, 2)          # [p,u,k_src,u_src,m,q]
        wi = t.reshape(128, U, KT, MT, 128)
    whp = w_hh[:, _PERM, :] * gs_row * 0.5          # *0.5: consumes h2
    wh = whp.reshape(U, MT, 128, KH, 128).transpose(4, 0, 3, 1, 2)
    bias = ((b_ih + b_hh)[:, _PERM]) * gs_row[:, 0]  # [U,1024]
    bias = bias.reshape(U, MT, 128).transpose(2, 0, 1).reshape(128, U * MT)
    return (np.ascontiguousarray(wi, dtype=BF16),
            np.ascontiguousarray(wh, dtype=BF16),
            np.ascontiguousarray(bias, dtype=np.float32))


def pack_dense(dense_w, dense_b, clf_w, clf_b):
    """dense rep feature order: [target 0:768 natural] + h1 tiles j=(k_src,u_src):
    col = 768 + u_src*256 + k_src*128 + p."""
    cols = []
    for kt in range(KD):
        p = np.arange(128)
        if kt < KT0:
            cols.append(kt * 128 + p)
        else:
            j = kt - KT0
            k_src, u_src = j // 2, j % 2
            cols.append(E + u_src * 256 + k_src * 128 + p)
    COL = np.stack(cols)                               # [10,128]
    dwT = dense_w.T[COL]                               # [10,128,640]
    dwT = dwT.copy()
    dwT[KT0:] *= 0.5                                   # h-columns consume h2=2h
    dw = dwT.reshape(KD, 128, MD, 128).transpose(1, 0, 2, 3)   # [128,10,5,128]
    db = dense_b.reshape(MD, 128).T                    # [128,5]
    cw = clf_w.T.reshape(MD, 128, 2).transpose(1, 0, 2)        # [128,5,2]
    cb = clf_b.reshape(2, 1)
    return (np.ascontiguousarray(dw, dtype=BF16),
            np.ascontiguousarray(db, dtype=np.float32),
            np.ascontiguousarray(cw, dtype=BF16),
            np.ascontiguousarray(cb, dtype=np.float32))


def host_pack(inputs):
    """All host-side preprocessing. Returns (per_core_inmaps_data, target_reps)."""
    emb = np.asarray(inputs["emb_table"], dtype=np.float32)
    article = np.asarray(inputs["article"])
    positions = np.asarray(inputs["positions"])

    looked = emb[article]                              # [B,T,E] fp32
    tgt = emb[article[np.arange(B), positions]]        # [B,E] fp32 (exact output)

    w0i, w0h, b0 = pack_lstm(np.asarray(inputs["w_ih0"], np.float32),
                             np.asarray(inputs["w_hh0"], np.float32),
                             np.asarray(inputs["b_ih0"], np.float32),
                             np.asarray(inputs["b_hh0"], np.float32), l1=False)
    w1i, w1h, b1 = pack_lstm(np.asarray(inputs["w_ih1"], np.float32),
                             np.asarray(inputs["w_hh1"], np.float32),
                             np.asarray(inputs["b_ih1"], np.float32),
                             np.asarray(inputs["b_hh1"], np.float32), l1=True)
    dw, db, cw, cb = pack_dense(np.asarray(inputs["dense_w"], np.float32),
                                np.asarray(inputs["dense_b"], np.float32),
                                np.asarray(inputs["clf_w"], np.float32),
                                np.asarray(inputs["clf_b"], np.float32))

    per_core = []
    for c in range(NCORES):
        bs = slice(c * BSH, (c + 1) * BSH)
        arr = looked[bs]                               # [b,T,768]
        xsT = arr.reshape(BSH, T, KT0, 128).transpose(3, 2, 1, 0)  # [128,6,T,b]
        tgtT = tgt[bs].reshape(BSH, KT0, 128).transpose(2, 1, 0)   # [128,6,b]
        per_core.append({
            "xsT": np.ascontiguousarray(xsT, dtype=BF16),
            "tgtT": np.ascontiguousarray(tgtT, dtype=BF16),
            "w0i": w0i, "w0h": w0h, "b0": b0,
            "w1i": w1i, "w1h": w1h, "b1": b1,
            "dw": dw, "db": db, "cw": cw, "cb": cb,
        })
    return per_core, tgt


# ---------------------------------------------------------------------------
# numpy simulation of the exact device dataflow (for fast host-side checking)
# ---------------------------------------------------------------------------
def simulate(inputs):
    per_core, tgt = host_pack(inputs)
    logits = np.zeros((B, 2), np.float32)
    probs = np.zeros((B, 2), np.float32)
    for c in range(NCORES):
        d = {k: np.asarray(v, np.float32) for k, v in per_core[c].items()}
        xsT = d["xsT"]                                 # [128,6,T,b]

        def proj(wi, KT, rhs_fn, bias):
            # returns projbig [128, U*MT(tcu), T, b] fp32
            out = np.zeros((128, U * MT, T, BSH), np.float32)
            for u in range(U):
                for m in range(MT):
                    acc = np.zeros((128, T * BSH), np.float32)
                    for k in range(KT):
                        lhsT = d[wi][:, u, k, m, :]    # [128,128] = [K,M]
                        rhs = rhs_fn(k)                # [128, T*b]
                        acc += lhsT.T @ rhs
                    out[:, m * U + u] = (acc + bias[:, u * MT + m][:, None]).reshape(128, T, BSH)
            return out

        def rec(projbig, wh):
            hist = np.zeros((128, T, KH, U, BSH), np.float32)
            ct = np.zeros((128, KH, U, BSH), np.float32)
            for t in range(T):
                psG = np.zeros((128, U * MT, BSH), np.float32)
                for u in range(U):
                    for m in range(MT):
                        for k in range(KH):
                            lhsT = d[wh][:, u, k, m, :]
                            rhs = (np.zeros((128, BSH), np.float32) if t == 0
                                   else hist[:, t - 1, k, u, :])
                            psG[:, m * U + u] += lhsT.T @ rhs
                gs = psG + projbig[:, :, t, :]
                th = np.tanh(gs).reshape(128, 4, KH * U, BSH)  # [p,type,(k,u),b]
                ctf = ct.reshape(128, KH * U, BSH)             # c2 = 2c
                A = (th[:, 1] + 1) * ctf                       # (th_f+1)*c2
                Bv = (th[:, 0] + 1) * th[:, 3]                 # (th_i+1)*th_g
                ctf[:] = A * 0.5 + Bv                          # c2' = 2c'
                tc2 = np.tanh(ctf * 0.5)                       # tanh(c)
                h2 = (th[:, 2] + 1) * tc2                      # 2h
                hist[:, t] = h2.reshape(128, KH, U, BSH)
            return hist

        proj0 = proj("w0i", KT0, lambda k: xsT[:, k].reshape(128, T * BSH), d["b0"])
        hist0 = rec(proj0, "w0h")
        proj1 = proj("w1i", KT1,
                     lambda j: hist0[:, :, j // 2, j % 2, :].reshape(128, T * BSH),
                     d["b1"])
        hist1 = rec(proj1, "w1h")

        # dense
        psD = np.zeros((128, MD, BSH), np.float32)
        for mt in range(MD):
            for kt in range(KD):
                lhsT = d["dw"][:, kt, mt, :]
                rhs = (d["tgtT"][:, kt, :] if kt < KT0
                       else hist1[:, T - 1, (kt - KT0) // 2, (kt - KT0) % 2, :])
                psD[:, mt] += lhsT.T @ rhs
        feats = np.tanh(psD + d["db"].reshape(128, MD, 1))
        psL = np.zeros((2, BSH), np.float32)
        for kt in range(MD):
            psL += d["cw"][:, kt, :].T @ feats[:, kt, :]
        lg = psL + d["cb"]
        logits[c * BSH:(c + 1) * BSH] = lg.T
        probs[c * BSH:(c + 1) * BSH] = (1 / (1 + np.exp(-lg))).T
    return logits, probs, tgt


# ---------------------------------------------------------------------------
# Bass program
# ---------------------------------------------------------------------------
_PROG_CACHE = {}


def _build_program():
    if "nc" in _PROG_CACHE:
        return _PROG_CACHE["nc"]
    import concourse.bass as bass
    import concourse.tile as tile
    from concourse import bacc, mybir

    f32 = mybir.dt.float32
    bf16 = mybir.dt.bfloat16
    AF = mybir.ActivationFunctionType
    ALU = mybir.AluOpType

    nc = bacc.Bacc("TRN2", target_bir_lowering=False, debug=False,
                   num_devices=NCORES)

    dr = {}
    def din(name, shape, dt):
        dr[name] = nc.dram_tensor(name, list(shape), dt, kind="ExternalInput").ap()

    din("xsT", (128, KT0, T, BSH), bf16)
    din("tgtT", (128, KT0, BSH), bf16)
    din("w0i", (128, U, KT0, MT, 128), bf16)
    din("w0h", (128, U, KH, MT, 128), bf16)
    din("b0", (128, U * MT), f32)
    din("w1i", (128, U, KT1, MT, 128), bf16)
    din("w1h", (128, U, KH, MT, 128), bf16)
    din("b1", (128, U * MT), f32)
    din("dw", (128, KD, MD, 128), bf16)
    din("db", (128, MD), f32)
    din("cw", (128, MD, 2), bf16)
    din("cb", (2, 1), f32)
    out_lg = nc.dram_tensor("out_logitsT", [2, BSH], f32, kind="ExternalOutput").ap()
    out_pr = nc.dram_tensor("out_probsT", [2, BSH], f32, kind="ExternalOutput").ap()

    with tile.TileContext(nc) as tc:
        with tc.tile_pool(name="const", bufs=1) as cp, \
             tc.tile_pool(name="projp", bufs=1) as projp, \
             tc.tile_pool(name="ew", bufs=3) as ew, \
             tc.tile_pool(name="pp", bufs=2, space=bass.MemorySpace.PSUM) as pp, \
             tc.tile_pool(name="gp", bufs=2, space=bass.MemorySpace.PSUM) as gp:

            sb = {}
            for name, ap in dr.items():
                t_ = cp.tile(list(ap.shape), ap.dtype, tag=name)
                nc.sync.dma_start(t_[:], ap[:])
                sb[name] = t_

            z = cp.tile([128, KH, U, BSH], bf16, tag="zeros")
            nc.gpsimd.memset(z[:], 0.0)

            proj0 = projp.tile([128, U * MT, T, BSH], f32, tag="proj0")
            proj1 = projp.tile([128, U * MT, T, BSH], f32, tag="proj1")
            hist0 = projp.tile([128, T, KH, U, BSH], bf16, tag="hist0")
            hist1 = projp.tile([128, T, KH, U, BSH], bf16, tag="hist1")
            c0 = projp.tile([128, KH * U, BSH], f32, tag="c0")
            c1 = projp.tile([128, KH * U, BSH], f32, tag="c1")
            nc.gpsimd.memset(c0[:], 0.0)
            nc.gpsimd.memset(c1[:], 0.0)

        # ---- input projections ----
            def do_proj(wi_name, KT, rhs_fn, bias_name, projbig):
                wi = sb[wi_name]
                for u in range(U):
                    for m in range(MT):
                        ps = pp.tile([128, T, BSH], f32, tag="pproj")
                        for k in range(KT):
                            nc.tensor.matmul(ps[:], wi[:, u, k, m, :], rhs_fn(k),
                                             start=(k == 0), stop=(k == KT - 1))
                        nc.scalar.activation(
                            projbig[:, m * U + u, :, :], ps[:], AF.Identity,
                            bias=sb[bias_name][:, u * MT + m: u * MT + m + 1])

            do_proj("w0i", KT0, lambda k: sb["xsT"][:, k, :, :], "b0", proj0)

            # ---- recurrence ----
            def do_rec(projbig, wh_name, hist, ct):
                wh = sb[wh_name]
                for t in range(T):
                    psG = gp.tile([128, U * MT, BSH], f32, tag="psG")
                    for u in range(U):
                        for m in range(MT):
                            for k in range(KH):
                                rhs = (z[:, k, u, :] if t == 0
                                       else hist[:, t - 1, k, u, :])
                                nc.tensor.matmul(psG[:, m * U + u, :],
                                                 wh[:, u, k, m, :], rhs,
                                                 start=(k == 0), stop=(k == KH - 1))
                    KU = KH * U
                    gs = ew.tile([128, U * MT, BSH], f32, tag="gs")
                    nc.vector.tensor_add(gs[:], psG[:], projbig[:, :, t, :])
                    # one tanh for all gates (i,f,o rows pre-scaled by 0.5)
                    th = ew.tile([128, U * MT, BSH], f32, tag="th")
                    nc.scalar.activation(th[:], gs[:], AF.Tanh)
                    # type-major slices: i=0:KU, f=KU:2KU, o=2KU:3KU, g=3KU:4KU
                    A = ew.tile([128, KU, BSH], f32, tag="A")
                    nc.vector.scalar_tensor_tensor(
                        A[:], th[:, KU:2 * KU, :], 1.0, ct[:],
                        ALU.add, ALU.mult)                   # (th_f+1)*c2
                    Bv = ew.tile([128, KU, BSH], f32, tag="Bv")
                    nc.vector.scalar_tensor_tensor(
                        Bv[:], th[:, 0:KU, :], 1.0, th[:, 3 * KU:, :],
                        ALU.add, ALU.mult)                   # (th_i+1)*th_g
                    nc.vector.scalar_tensor_tensor(
                        ct[:], A[:], 0.5, Bv[:], ALU.mult, ALU.add)  # c2'
                    tc2 = ew.tile([128, KU, BSH], f32, tag="tc2")
                    nc.scalar.activation(tc2[:], ct[:], AF.Tanh, scale=0.5)
                    nc.vector.scalar_tensor_tensor(
                        hist[:, t, :, :, :], th[:, 2 * KU:3 * KU, :], 1.0,
                        tc2[:], ALU.add, ALU.mult)           # h2 = (th_o+1)*tanh(c)

            do_rec(proj0, "w0h", hist0, c0)
            do_proj("w1i", KT1,
                    lambda j: hist0[:, :, j // 2, j % 2, :], "b1", proj1)
            do_rec(proj1, "w1h", hist1, c1)

            # ---- dense + classifier ----
            psD = gp.tile([128, MD, BSH], f32, tag="psD")
            for mt in range(MD):
                for kt in range(KD):
                    rhs = (sb["tgtT"][:, kt, :] if kt < KT0
                           else hist1[:, T - 1, (kt - KT0) // 2, (kt - KT0) % 2, :])
                    nc.tensor.matmul(psD[:, mt, :], sb["dw"][:, kt, mt, :], rhs,
                                     start=(kt == 0), stop=(kt == KD - 1))
            feats = ew.tile([128, MD, BSH], bf16, tag="feats")
            for mt in range(MD):
                nc.scalar.activation(feats[:, mt, :], psD[:, mt, :], AF.Tanh,
                                     bias=sb["db"][:, mt:mt + 1])
            psL = gp.tile([2, BSH], f32, tag="psL")
            for kt in range(MD):
                nc.tensor.matmul(psL[:], sb["cw"][:, kt, :], feats[:, kt, :],
                                 start=(kt == 0), stop=(kt == MD - 1))
            lgt = ew.tile([2, BSH], f32, tag="lgt")
            nc.scalar.activation(lgt[:], psL[:], AF.Identity, bias=sb["cb"][:, 0:1])
            prt = ew.tile([2, BSH], f32, tag="prt")
            nc.scalar.activation(prt[:], psL[:], AF.Sigmoid, bias=sb["cb"][:, 0:1])
            nc.sync.dma_start(out_lg[:], lgt[:])
            nc.sync.dma_start(out_pr[:], prt[:])

    nc.compile()
    _PROG_CACHE["nc"] = nc
    return nc


def kernel(**inputs):
    per_core, tgt = host_pack(inputs)
    nc = _build_program()
    from concourse.bass_utils import run_bass_kernel_spmd
    res = run_bass_kernel_spmd(nc, per_core, core_ids=list(range(NCORES)),
                               trace=False)
    logits = np.zeros((B, 2), np.float32)
    probs = np.zeros((B, 2), np.float32)
    for c in range(NCORES):
        logits[c * BSH:(c + 1) * BSH] = res.results[c]["out_logitsT"].T
        probs[c * BSH:(c + 1) * BSH] = res.results[c]["out_probsT"].T
    return logits, probs, tgt.astype(np.float32)


# revision 14
# speedup vs baseline: 1.3741x; 1.3368x over previous
"""Trainium2 Bass kernel for nn_ContextAwareModel (2-layer 'bidirectional' LSTM
over gathered sentence embeddings + dense head).

Strategy (hardcoded, self-contained):
  - Host: gather embedding rows (sparse lookup of 4096+32 rows from the 307MB
    table stays on host - only 12.6MB of gathered data ships to devices),
    pre-transpose/pack all tensors into PE-friendly tiles.
  - Device (SPMD over 8 cores, data-parallel over batch, b=4 per core):
    L0 input projections (big matmuls) -> L0 recurrence (128 steps, both
    directions) -> L1 projections -> L1 recurrence -> dense+classifier.
    All matmul inputs bf16 (FWL weight loads), fp32 PSUM/state.
  - Layout: gates on partitions [128 = hidden-in-chunk], batch on free dim.
    Gate-type order (i, f, o, g) so one sigmoid covers a contiguous slice.
"""

import numpy as np
import ml_dtypes

E = 768
H = 256
B = 32
T = 128
D = E + 2 * H
HALF = D // 2
NCORES = 8
import os as _os
MODE = _os.environ.get("KMODE", "v1")   # v1: data-parallel b=4, all 4 units/core
                                        # v3: 4 quarters x 2 members, 1 unit/phase/core + pair AllGather
U = 2                      # directions per layer
BSH = B // NCORES if MODE == "v1" else B // 4   # batch per core
UL = U if MODE == "v1" else 1                   # units computed locally per layer
KT0 = E // 128             # 6  K-tiles for L0 input proj
KT1 = (2 * H) // 128       # 4  K-tiles for L1 input proj
KH = H // 128              # 2  K-tiles for recurrent matmul
MT = (4 * H) // 128        # 8  M-tiles of gates per direction
KD = D // 128              # 10 K-tiles for dense
MD = HALF // 128           # 5  M-tiles for dense
# our gate-type order (i,f,o,g) -> pytorch row-block (i,f,g,o)
TP_MAP = [0, 1, 3, 2]

BF16 = ml_dtypes.bfloat16


def _perm_rows():
    """Row permutation: our M-tile m covers pytorch rows TP_MAP[m//2]*256+(m%2)*128."""
    idx = []
    for m in range(MT):
        base = TP_MAP[m // 2] * 256 + (m % 2) * 128
        idx.append(np.arange(base, base + 128))
    return np.concatenate(idx)          # [1024]


_PERM = _perm_rows()


def pack_lstm(w_ih, w_hh, b_ih, b_hh, l1: bool):
    """Pack one layer's weights.
    Returns wi [128,U,KT,MT,128] bf16, wh [128,U,KH,MT,128] bf16, bias [128,U*MT] f32.
    lhsT tile convention: tile[p, q] = W[row(m,q), col(k,p)]  (pre-transposed).
    For L1 the input features are ordered (k_src, u_src, p) to match the
    on-chip h-history layout, i.e. col(j,p) = u_src*256 + k_src*128 + p with
    j = k_src*2 + u_src.
    """
    # Gate-row scaling for the fused one-tanh cell update:
    #   sigmoid(x) = (tanh(x/2)+1)/2  -> scale i,f,o gate rows by 0.5 so one
    #   Tanh activation serves all four gates.
    #   h is stored as h2 = 2h -> scale all h-consuming weight columns by 0.5
    #   (w_hh here; w_ih1 and dense h-columns elsewhere).
    gs_row = np.ones((4 * H, 1), np.float32)
    for m in range(MT):
        if m // 2 < 3:                  # our types i,f,o
            gs_row[m * 128:(m + 1) * 128] = 0.5
    wp = w_ih[:, _PERM, :] * gs_row     # [U,1024,IN]
    if l1:
        wp = wp * 0.5                   # consumes h2 from layer 0
    KT = wp.shape[2] // 128
    if not l1:
        wi = wp.reshape(U, MT, 128, KT, 128).transpose(4, 0, 3, 1, 2)
    else:
        # IN=512 -> [u_src,k_src,p]; want K-tile j=(k_src,u_src)
        t = wp.reshape(U, MT, 128, 2, 2, 128)      # [u,m,q,u_src,k_src,p]
        t = t.transpose(5, 0, 4, 3, 1, 2)          # [p,u,k_src,u_src,m,q]
        wi = t.reshape(128, U, KT, MT, 128)
    whp = w_hh[:, _PERM, :] * gs_row * 0.5          # *0.5: consumes h2
    wh = whp.reshape(U, MT, 128, KH, 128).transpose(4, 0, 3, 1, 2)
    bias = ((b_ih + b_hh)[:, _PERM]) * gs_row[:, 0]  # [U,1024]
    bias = bias.reshape(U, MT, 128).transpose(2, 0, 1).reshape(128, U * MT)
    return (np.ascontiguousarray(wi, dtype=BF16),
            np.ascontiguousarray(wh, dtype=BF16),
            np.ascontiguousarray(bias, dtype=np.float32))


def pack_dense(dense_w, dense_b, clf_w, clf_b):
    """dense rep feature order: [target 0:768 natural] + h1 tiles j=(k_src,u_src):
    col = 768 + u_src*256 + k_src*128 + p."""
    cols = []
    for kt in range(KD):
        p = np.arange(128)
        if kt < KT0:
            cols.append(kt * 128 + p)
        else:
            j = kt - KT0
            k_src, u_src = j // 2, j % 2
            cols.append(E + u_src * 256 + k_src * 128 + p)
    COL = np.stack(cols)                               # [10,128]
    dwT = dense_w.T[COL]                               # [10,128,640]
    dwT = dwT.copy()
    dwT[KT0:] *= 0.5                                   # h-columns consume h2=2h
    dw = dwT.reshape(KD, 128, MD, 128).transpose(1, 0, 2, 3)   # [128,10,5,128]
    db = dense_b.reshape(MD, 128).T                    # [128,5]
    cw = clf_w.T.reshape(MD, 128, 2).transpose(1, 0, 2)        # [128,5,2]
    cb = clf_b.reshape(2, 1)
    return (np.ascontiguousarray(dw, dtype=BF16),
            np.ascontiguousarray(db, dtype=np.float32),
            np.ascontiguousarray(cw, dtype=BF16),
            np.ascontiguousarray(cb, dtype=np.float32))


def host_pack(inputs):
    """All host-side preprocessing. Returns (per_core_inmaps_data, target_reps)."""
    emb = np.asarray(inputs["emb_table"], dtype=np.float32)
    article = np.asarray(inputs["article"])
    positions = np.asarray(inputs["positions"])

    looked = emb[article]                              # [B,T,E] fp32
    tgt = emb[article[np.arange(B), positions]]        # [B,E] fp32 (exact output)

    w0i, w0h, b0 = pack_lstm(np.asarray(inputs["w_ih0"], np.float32),
                             np.asarray(inputs["w_hh0"], np.float32),
                             np.asarray(inputs["b_ih0"], np.float32),
                             np.asarray(inputs["b_hh0"], np.float32), l1=False)
    w1i, w1h, b1 = pack_lstm(np.asarray(inputs["w_ih1"], np.float32),
                             np.asarray(inputs["w_hh1"], np.float32),
                             np.asarray(inputs["b_ih1"], np.float32),
                             np.asarray(inputs["b_hh1"], np.float32), l1=True)
    dw, db, cw, cb = pack_dense(np.asarray(inputs["dense_w"], np.float32),
                                np.asarray(inputs["dense_b"], np.float32),
                                np.asarray(inputs["clf_w"], np.float32),
                                np.asarray(inputs["clf_b"], np.float32))

    per_core = []
    for c in range(NCORES):
        if MODE == "v1":
            bs = slice(c * BSH, (c + 1) * BSH)
            wsl = {"w0i": w0i, "w0h": w0h, "b0": b0,
                   "w1i": w1i, "w1h": w1h, "b1": b1}
        else:
            q, r = c // 2, c % 2
            bs = slice(q * BSH, (q + 1) * BSH)
            wsl = {"w0i": np.ascontiguousarray(w0i[:, r:r + 1]),
                   "w0h": np.ascontiguousarray(w0h[:, r:r + 1]),
                   "b0": np.ascontiguousarray(b0[:, r * MT:(r + 1) * MT]),
                   "w1i": np.ascontiguousarray(w1i[:, r:r + 1]),
                   "w1h": np.ascontiguousarray(w1h[:, r:r + 1]),
                   "b1": np.ascontiguousarray(b1[:, r * MT:(r + 1) * MT])}
        arr = looked[bs]                               # [b,T,768]
        xsT = arr.reshape(BSH, T, KT0, 128).transpose(3, 2, 1, 0)  # [128,6,T,b]
        tgtT = tgt[bs].reshape(BSH, KT0, 128).transpose(2, 1, 0)   # [128,6,b]
        per_core.append({
            "xsT": np.ascontiguousarray(xsT, dtype=BF16),
            "tgtT": np.ascontiguousarray(tgtT, dtype=BF16),
            **wsl,
            "dw": dw, "db": db, "cw": cw, "cb": cb,
        })
    return per_core, tgt


# ---------------------------------------------------------------------------
# numpy simulation of the exact device dataflow (for fast host-side checking)
# ---------------------------------------------------------------------------
def simulate(inputs):
    per_core, tgt = host_pack(inputs)
    logits = np.zeros((B, 2), np.float32)
    probs = np.zeros((B, 2), np.float32)
    for c in range(NCORES):
        d = {k: np.asarray(v, np.float32) for k, v in per_core[c].items()}
        xsT = d["xsT"]                                 # [128,6,T,b]

        def proj(wi, KT, rhs_fn, bias):
            # returns projbig [128, U*MT(tcu), T, b] fp32
            out = np.zeros((128, U * MT, T, BSH), np.float32)
            for u in range(U):
                for m in range(MT):
                    acc = np.zeros((128, T * BSH), np.float32)
                    for k in range(KT):
                        lhsT = d[wi][:, u, k, m, :]    # [128,128] = [K,M]
                        rhs = rhs_fn(k)                # [128, T*b]
                        acc += lhsT.T @ rhs
                    out[:, m * U + u] = (acc + bias[:, u * MT + m][:, None]).reshape(128, T, BSH)
            return out

        def rec(projbig, wh):
            hist = np.zeros((128, T, KH, U, BSH), np.float32)
            ct = np.zeros((128, KH, U, BSH), np.float32)
            for t in range(T):
                psG = np.zeros((128, U * MT, BSH), np.float32)
                for u in range(U):
                    for m in range(MT):
                        for k in range(KH):
                            lhsT = d[wh][:, u, k, m, :]
                            rhs = (np.zeros((128, BSH), np.float32) if t == 0
                                   else hist[:, t - 1, k, u, :])
                            psG[:, m * U + u] += lhsT.T @ rhs
                gs = psG + projbig[:, :, t, :]
                th = np.tanh(gs).reshape(128, 4, KH * U, BSH)  # [p,type,(k,u),b]
                ctf = ct.reshape(128, KH * U, BSH)             # c2 = 2c
                A = (th[:, 1] + 1) * ctf                       # (th_f+1)*c2
                Bv = (th[:, 0] + 1) * th[:, 3]                 # (th_i+1)*th_g
                ctf[:] = A * 0.5 + Bv                          # c2' = 2c'
                tc2 = np.tanh(ctf * 0.5)                       # tanh(c)
                h2 = (th[:, 2] + 1) * tc2                      # 2h
                hist[:, t] = h2.reshape(128, KH, U, BSH)
            return hist

        proj0 = proj("w0i", KT0, lambda k: xsT[:, k].reshape(128, T * BSH), d["b0"])
        hist0 = rec(proj0, "w0h")
        proj1 = proj("w1i", KT1,
                     lambda j: hist0[:, :, j // 2, j % 2, :].reshape(128, T * BSH),
                     d["b1"])
        hist1 = rec(proj1, "w1h")

        # dense
        psD = np.zeros((128, MD, BSH), np.float32)
        for mt in range(MD):
            for kt in range(KD):
                lhsT = d["dw"][:, kt, mt, :]
                rhs = (d["tgtT"][:, kt, :] if kt < KT0
                       else hist1[:, T - 1, (kt - KT0) // 2, (kt - KT0) % 2, :])
                psD[:, mt] += lhsT.T @ rhs
        feats = np.tanh(psD + d["db"].reshape(128, MD, 1))
        psL = np.zeros((2, BSH), np.float32)
        for kt in range(MD):
            psL += d["cw"][:, kt, :].T @ feats[:, kt, :]
        lg = psL + d["cb"]
        logits[c * BSH:(c + 1) * BSH] = lg.T
        probs[c * BSH:(c + 1) * BSH] = (1 / (1 + np.exp(-lg))).T
    return logits, probs, tgt


# ---------------------------------------------------------------------------
# Bass program
# ---------------------------------------------------------------------------
_PROG_CACHE = {}


def _build_program():
    if "nc" in _PROG_CACHE:
        return _PROG_CACHE["nc"]
    import concourse.bass as bass
    import concourse.tile as tile
    from concourse import bacc, mybir

    f32 = mybir.dt.float32
    bf16 = mybir.dt.bfloat16
    AF = mybir.ActivationFunctionType
    ALU = mybir.AluOpType

    nc = bacc.Bacc("TRN2", target_bir_lowering=False, debug=False,
                   num_devices=NCORES)

    dr = {}
    def din(name, shape, dt):
        dr[name] = nc.dram_tensor(name, list(shape), dt, kind="ExternalInput").ap()

    din("xsT", (128, KT0, T, BSH), bf16)
    din("tgtT", (128, KT0, BSH), bf16)
    din("w0i", (128, UL, KT0, MT, 128), bf16)
    din("w0h", (128, UL, KH, MT, 128), bf16)
    din("b0", (128, UL * MT), f32)
    din("w1i", (128, UL, KT1, MT, 128), bf16)
    din("w1h", (128, UL, KH, MT, 128), bf16)
    din("b1", (128, UL * MT), f32)
    din("dw", (128, KD, MD, 128), bf16)
    din("db", (128, MD), f32)
    din("cw", (128, MD, 2), bf16)
    din("cb", (2, 1), f32)
    out_lg = nc.dram_tensor("out_logitsT", [2, BSH], f32, kind="ExternalOutput").ap()
    out_pr = nc.dram_tensor("out_probsT", [2, BSH], f32, kind="ExternalOutput").ap()

    with tile.TileContext(nc) as tc:
        with tc.tile_pool(name="const", bufs=1) as cp, \
             tc.tile_pool(name="projp", bufs=1) as projp, \
             tc.tile_pool(name="ew", bufs=3) as ew, \
             tc.tile_pool(name="pp", bufs=2, space=bass.MemorySpace.PSUM) as pp, \
             tc.tile_pool(name="gp", bufs=2, space=bass.MemorySpace.PSUM) as gp:

            sb = {}
            for name, ap in dr.items():
                t_ = cp.tile(list(ap.shape), ap.dtype, tag=name)
                nc.sync.dma_start(t_[:], ap[:])
                sb[name] = t_

            z = cp.tile([128, KH, UL, BSH], bf16, tag="zeros")
            nc.gpsimd.memset(z[:], 0.0)

            proj0 = projp.tile([128, UL * MT, T, BSH], f32, tag="proj0")
            proj1 = projp.tile([128, UL * MT, T, BSH], f32, tag="proj1")
            hist0 = projp.tile([128, T, KH, UL, BSH], bf16, tag="hist0")
            hist1 = projp.tile([128, T, KH, UL, BSH], bf16, tag="hist1")
            c0 = projp.tile([128, KH * UL, BSH], f32, tag="c0")
            c1 = projp.tile([128, KH * UL, BSH], f32, tag="c1")
            nc.gpsimd.memset(c0[:], 0.0)
            nc.gpsimd.memset(c1[:], 0.0)

            # proj PSUM tile must fit one bank: <=512 fp32 free elems
            TH = min(T, 512 // BSH)
            NHALF = T // TH

        # ---- input projections ----
            def do_proj(wi_name, KT, rhs_fn, bias_name, projbig):
                wi = sb[wi_name]
                for u in range(UL):
                    for m in range(MT):
                        for h in range(NHALF):
                            ps = pp.tile([128, TH, BSH], f32, tag="pproj")
                            for k in range(KT):
                                nc.tensor.matmul(ps[:], wi[:, u, k, m, :],
                                                 rhs_fn(k, h),
                                                 start=(k == 0), stop=(k == KT - 1))
                            nc.scalar.activation(
                                projbig[:, m * UL + u, h * TH:(h + 1) * TH, :],
                                ps[:], AF.Identity,
                                bias=sb[bias_name][:, u * MT + m: u * MT + m + 1])

            do_proj("w0i", KT0,
                    lambda k, h: sb["xsT"][:, k, h * TH:(h + 1) * TH, :],
                    "b0", proj0)

            # ---- recurrence ----
            def do_rec(projbig, wh_name, hist, ct):
                wh = sb[wh_name]
                for t in range(T):
                    psG = gp.tile([128, UL * MT, BSH], f32, tag="psG")
                    for u in range(UL):
                        for m in range(MT):
                            for k in range(KH):
                                rhs = (z[:, k, u, :] if t == 0
                                       else hist[:, t - 1, k, u, :])
                                nc.tensor.matmul(psG[:, m * UL + u, :],
                                                 wh[:, u, k, m, :], rhs,
                                                 start=(k == 0), stop=(k == KH - 1))
                    KU = KH * UL
                    gs = ew.tile([128, UL * MT, BSH], f32, tag="gs")
                    nc.vector.tensor_add(gs[:], psG[:], projbig[:, :, t, :])
                    # one tanh for all gates (i,f,o rows pre-scaled by 0.5)
                    th = ew.tile([128, UL * MT, BSH], f32, tag="th")
                    nc.scalar.activation(th[:], gs[:], AF.Tanh)
                    # type-major slices: i=0:KU, f=KU:2KU, o=2KU:3KU, g=3KU:4KU
                    A = ew.tile([128, KU, BSH], f32, tag="A")
                    nc.vector.scalar_tensor_tensor(
                        A[:], th[:, KU:2 * KU, :], 1.0, ct[:],
                        ALU.add, ALU.mult)                   # (th_f+1)*c2
                    Bv = ew.tile([128, KU, BSH], f32, tag="Bv")
                    nc.vector.scalar_tensor_tensor(
                        Bv[:], th[:, 0:KU, :], 1.0, th[:, 3 * KU:, :],
                        ALU.add, ALU.mult)                   # (th_i+1)*th_g
                    nc.vector.scalar_tensor_tensor(
                        ct[:], A[:], 0.5, Bv[:], ALU.mult, ALU.add)  # c2'
                    tc2 = ew.tile([128, KU, BSH], f32, tag="tc2")
                    nc.scalar.activation(tc2[:], ct[:], AF.Tanh, scale=0.5)
                    nc.vector.scalar_tensor_tensor(
                        hist[:, t, :, :, :], th[:, 2 * KU:3 * KU, :], 1.0,
                        tc2[:], ALU.add, ALU.mult)           # h2 = (th_o+1)*tanh(c)

            do_rec(proj0, "w0h", hist0, c0)

            if MODE == "v1":
                hist0f = hist0
            else:
                with tc.tile_pool(name="dramb", bufs=1, space="DRAM") as dp:
                    bin0 = dp.tile([128, T * KH * BSH], bf16, tag="agin0")
                    bout0 = dp.tile([256, T * KH * BSH], bf16, tag="agout0")
                    nc.sync.dma_start(bin0[:], hist0[:])
                    nc.gpsimd.collective_compute(
                        "AllGather", mybir.AluOpType.bypass,
                        replica_groups=[[0, 1], [2, 3], [4, 5], [6, 7]],
                        ins=[bin0.opt()], outs=[bout0.opt()])
                    hist0f = projp.tile([128, T, KH, U, BSH], bf16, tag="hist0f")
                    for u2 in range(U):
                        nc.sync.dma_start(hist0f[:, :, :, u2, :],
                                          bout0[u2 * 128:(u2 + 1) * 128, :])

            do_proj("w1i", KT1,
                    lambda j, h: hist0f[:, h * TH:(h + 1) * TH, j // 2, j % 2, :],
                    "b1", proj1)
            do_rec(proj1, "w1h", hist1, c1)

            if MODE == "v1":
                h1f = hist1[:, T - 1, :, :, :]
            else:
                with tc.tile_pool(name="dramb2", bufs=1, space="DRAM") as dp2:
                    bin1 = dp2.tile([128, KH * BSH], bf16, tag="agin1")
                    bout1 = dp2.tile([256, KH * BSH], bf16, tag="agout1")
                    nc.sync.dma_start(bin1[:], hist1[:, T - 1, :, :, :])
                    nc.gpsimd.collective_compute(
                        "AllGather", mybir.AluOpType.bypass,
                        replica_groups=[[0, 1], [2, 3], [4, 5], [6, 7]],
                        ins=[bin1.opt()], outs=[bout1.opt()])
                    h1full = projp.tile([128, KH, U, BSH], bf16, tag="h1full")
                    for u2 in range(U):
                        nc.sync.dma_start(h1full[:, :, u2, :],
                                          bout1[u2 * 128:(u2 + 1) * 128, :])
                    h1f = h1full[:, :, :, :]

            # ---- dense + classifier ----
            psD = gp.tile([128, MD, BSH], f32, tag="psD")
            for mt in range(MD):
                for kt in range(KD):
                    rhs = (sb["tgtT"][:, kt, :] if kt < KT0
                           else h1f[:, (kt - KT0) // 2, (kt - KT0) % 2, :])
                    nc.tensor.matmul(psD[:, mt, :], sb["dw"][:, kt, mt, :], rhs,
                                     start=(kt == 0), stop=(kt == KD - 1))
            feats = ew.tile([128, MD, BSH], bf16, tag="feats")
            for mt in range(MD):
                nc.scalar.activation(feats[:, mt, :], psD[:, mt, :], AF.Tanh,
                                     bias=sb["db"][:, mt:mt + 1])
            psL = gp.tile([2, BSH], f32, tag="psL")
            for kt in range(MD):
                nc.tensor.matmul(psL[:], sb["cw"][:, kt, :], feats[:, kt, :],
                                 start=(kt == 0), stop=(kt == MD - 1))
            lgt = ew.tile([2, BSH], f32, tag="lgt")
            nc.scalar.activation(lgt[:], psL[:], AF.Identity, bias=sb["cb"][:, 0:1])
            prt = ew.tile([2, BSH], f32, tag="prt")
            nc.scalar.activation(prt[:], psL[:], AF.Sigmoid, bias=sb["cb"][:, 0:1])
            nc.sync.dma_start(out_lg[:], lgt[:])
            nc.sync.dma_start(out_pr[:], prt[:])

    nc.compile()
    _PROG_CACHE["nc"] = nc
    return nc


def kernel(**inputs):
    per_core, tgt = host_pack(inputs)
    nc = _build_program()
    from concourse.bass_utils import run_bass_kernel_spmd
    res = run_bass_kernel_spmd(nc, per_core, core_ids=list(range(NCORES)),
                               trace=False)
    logits = np.zeros((B, 2), np.float32)
    probs = np.zeros((B, 2), np.float32)
    if MODE == "v1":
        for c in range(NCORES):
            logits[c * BSH:(c + 1) * BSH] = res.results[c]["out_logitsT"].T
            probs[c * BSH:(c + 1) * BSH] = res.results[c]["out_probsT"].T
    else:
        for q in range(4):
            logits[q * BSH:(q + 1) * BSH] = res.results[2 * q]["out_logitsT"].T
            probs[q * BSH:(q + 1) * BSH] = res.results[2 * q]["out_probsT"].T
    return logits, probs, tgt.astype(np.float32)
